# revision 63
# baseline (speedup 1.0000x reference)
"""Performer (FAVOR+) multi-head fast-attention TRN2 kernel — self-contained.

Problem: B=4, N=4096, D=1024, H=16, M=256, DH=64.
Sharding: 2 heads per core (head-parallel attention) on 8 NeuronCores;
on-device AllToAll re-shards to sequence-parallel for the output Linear
(row-parallel, no partial sums); host stitches the 8 n-shards.

All matmul traffic is bf16 (PE runs 1 col/cycle vs 4 for fp32-HIGH);
accumulation stays fp32 in PSUM.  Stabilizers that cancel in the
num/den ratio are dropped; the k-side row max and ||k||^2 factors are
folded into v, matching the reference up to float rounding.

Pipeline: the kernel runs as 8 "slots" (one per batch x head-pair).
The scalar engine (exp) is the binding resource, so each slot emits
its own k-feature exps first and the PREVIOUS slot's q-feature exps
second, keeping scalar 100% busy.  ctx/out matmuls of the previous
slot fill the PE pacing gaps; the per-slot AllToAll and the output
Linear of earlier batches are interleaved behind.
"""
import contextlib
import sys

sys.path.insert(0, "/opt/trn_rl_repo")

import numpy as np
import ml_dtypes

import concourse.bacc as bacc
import concourse.mybir as mybir
from concourse import library_config
from concourse.tile import TileContext
from concourse.bass_utils import run_bass_kernel_spmd
F32 = mybir.dt.float32
BF16 = mybir.dt.bfloat16
AF = mybir.ActivationFunctionType
ALU = mybir.AluOpType
NPBF16 = ml_dtypes.bfloat16

NCORES = 8
B, N, D = 4, 4096, 1024
H, M, DH = 16, 256, 64
T = N // 128          # 32 token tiles of 128
J = N // 512          # 8 query blocks of 512
NS = N // NCORES      # 512 tokens per core after resharding
DS = float(DH) ** -0.25

_CACHE = {}


def _build():
    nc = bacc.Bacc(num_devices=NCORES)
    groups = [list(range(NCORES))]

    qkT = nc.declare_dram_parameter("qkT", [B, 2, 128, N], BF16, isOutput=False)
    knvn = nc.declare_dram_parameter("knvn", [B, 128, T, 256], BF16, isOutput=False)
    projKZ = nc.declare_dram_parameter("projKZ", [128, M], BF16, isOutput=False)
    projQZ = nc.declare_dram_parameter("projQZ", [128, M], BF16, isOutput=False)
    WT = nc.declare_dram_parameter("WT", [128, NCORES, D], BF16, isOutput=False)
    ident = nc.declare_dram_parameter("ident", [128, 128], F32, isOutput=False)
    out_ext = nc.declare_dram_parameter("out", [B, NS, D], F32, isOutput=True)

    h_in = nc.dram_tensor("h_in", [B, NCORES, 2, 64, NS], BF16)
    h_out = nc.dram_tensor("h_out", [B, NCORES, 2, 64, NS], BF16)
    warm_in = nc.dram_tensor("warm_in", [NCORES, 64], BF16)
    warm_out = nc.dram_tensor("warm_out", [NCORES, 64], BF16)

    with TileContext(nc) as tc:
        with contextlib.ExitStack() as stk:
            const_p = stk.enter_context(tc.tile_pool(name="const", bufs=1))
            qkT_p = stk.enter_context(tc.tile_pool(name="qkT", bufs=3))
            knvn_p = stk.enter_context(tc.tile_pool(name="knvn", bufs=2))
            ek_p = stk.enter_context(tc.tile_pool(name="ek", bufs=2))
            small_p = stk.enter_context(tc.tile_pool(name="small", bufs=2))
            vaug_p = stk.enter_context(tc.tile_pool(name="vaug", bufs=2))
            qpt_p = stk.enter_context(tc.tile_pool(name="qpt", bufs=4))
            stg_p = stk.enter_context(tc.tile_pool(name="stg", bufs=2))
            hgn_p = stk.enter_context(tc.tile_pool(name="hgn", bufs=2))
            oc_p = stk.enter_context(tc.tile_pool(name="oc", bufs=1))
            # PSUM budget (8 banks): mm 2x2 (k- and q-feature tiles,
            # double buffered) + ctx 1 + po 1 + pl 2x1; transposes
            # borrow mm tiles.
            ps_mm = stk.enter_context(tc.tile_pool(name="psmm", bufs=2, space="PSUM"))
            ps_ctx = stk.enter_context(tc.tile_pool(name="psctx", bufs=1, space="PSUM"))
            ps_po = stk.enter_context(tc.tile_pool(name="pspo", bufs=2, space="PSUM"))
            ps_pl = stk.enter_context(tc.tile_pool(name="pspl", bufs=1, space="PSUM"))

            # -- warmup collective: primes the CC rings / absorbs core skew
            warm_sb = const_p.tile([NCORES, 64], BF16, tag="warm")
            nc.gpsimd.memset(warm_sb[:], 0.0)
            nc.sync.dma_start(out=warm_in[:], in_=warm_sb[:])
            nc.gpsimd.collective_compute(
                "AllToAll", ALU.bypass, replica_groups=groups,
                ins=[warm_in[:]], outs=[warm_out[:]])
            # pin the one gpsimd library that has BOTH tensor_tensor and
            # partition_broadcast: avoids ~7us lib reloads on every switch
            nc.gpsimd.load_library(library_config.proxy)



            state = {}

            def emit_knvn_load(b, chunked=False):
                # state holds 4 chunk views [128, 8, 256]; for the first
                # batch they are separate tiles so dependencies resolve
                # per chunk instead of per whole-tile
                if chunked:
                    views = []
                    for c in range(4):
                        cv = const_p.tile([128, 8, 256], BF16, tag=f"knvnc{c}")
                        for pp in range(2):
                            nc.sync.dma_start(
                                out=cv[64 * pp:64 * (pp + 1), :, :],
                                in_=knvn[b, 64 * pp:64 * (pp + 1),
                                         8 * c:8 * (c + 1), :])
                        views.append(cv)
                else:
                    knvn_sb = knvn_p.tile([128, T, 256], BF16, tag="knvn")
                    for pp in range(4):
                        nc.sync.dma_start(
                            out=knvn_sb[32 * pp:32 * (pp + 1), :, :],
                            in_=knvn[b, 32 * pp:32 * (pp + 1), :, :])
                    views = [knvn_sb[:, 8 * c:8 * (c + 1), :]
                             for c in range(4)]
                state[("knvn", b)] = views

            def emit_ksq_dn(b):
                # chunked so slot-open dn/eg pieces unblock one by one
                for cv in state[("knvn", b)]:
                    ksl = cv[:, :, 0:128]
                    nc.gpsimd.tensor_tensor(out=ksl, in0=ksl, in1=ksl,
                                            op=ALU.mult)

            def emit_qkT_load(b, h, chunked=False):
                # state holds 8 column-chunk views [128, 512]; the first
                # slot uses separate tiles for per-chunk dependencies
                if chunked:
                    views = []
                    for cb in range(8):
                        cv = const_p.tile([128, 512], BF16,
                                          tag=f"qkTc{h}_{cb}")
                        for pp in range(2):
                            nc.sync.dma_start(
                                out=cv[64 * pp:64 * (pp + 1), :],
                                in_=qkT[b, h, 64 * pp:64 * (pp + 1),
                                        512 * cb:512 * (cb + 1)])
                        views.append(cv)
                else:
                    qkT_sb = qkT_p.tile([128, N], BF16, tag="qkT")
                    for pp in range(2):
                        nc.sync.dma_start(
                            out=qkT_sb[64 * pp:64 * (pp + 1), :],
                            in_=qkT[b, h, 64 * pp:64 * (pp + 1), :])
                    views = [qkT_sb[:, 512 * cb:512 * (cb + 1)]
                             for cb in range(8)]
                state[("qkT", b, h)] = views

            def emit_post_dma(b, dma_eng=None, heads=(0, 1)):
                # After the AllToAlls of batch b: gather the (already
                # normalized) numerators -> ready for the Linear.
                eng = dma_eng if dma_eng is not None else nc.sync
                if ("hgn", b) in state:
                    hgn = state[("hgn", b)]
                else:
                    hgn = hgn_p.tile([128, NCORES, NS], BF16, tag="hgn")
                    state[("hgn", b)] = hgn
                for hh in heads:
                    eng.dma_start(
                        out=hgn[DH * hh:DH * (hh + 1), :, :],
                        in_=h_out[b, :, hh].rearrange("c d n -> d c n"))

            def emit_lin_group(b, g):
                # one PSUM accumulation group of the output Linear of batch b
                hgn = state[("hgn", b)]
                nci, oh = g // 2, g % 2
                if oh == 0:
                    oc_new = oc_p.tile([128, 2, 512], F32, tag="oc", name="oc")
                    state[("oc", b, nci)] = oc_new
                oc = state[("oc", b, nci)]
                pl = ps_pl.tile([128, 512], F32, tag="pl")
                for cc in range(NCORES):
                    nc.tensor.matmul(
                        pl[:], hgn[:, cc, 128 * nci:128 * (nci + 1)],
                        WT_sb[:, cc, 512 * oh:512 * (oh + 1)],
                        start=(cc == 0), stop=(cc == NCORES - 1),
                        skip_group_check=True)
                nc.vector.tensor_copy(oc[:, oh, :], pl[:])
                if oh == 1:
                    nc.sync.dma_start(
                        out=out_ext[b, 128 * nci:128 * (nci + 1), :],
                        in_=oc[:].rearrange("p a f -> p (a f)"))

            # ---- per-slot pieces -------------------------------------
            def emit_kf_step(s, tb):
                # 4 k-feature MMs into a feat tile + exp + me chain
                b, h = s
                qkT_sb = state[("qkT", b, h)]
                ek = state[("ek", s)]
                me = state[("me", s)]
                pf = ps_mm.tile([128, 2, 512], F32, tag="mm", name="pf")
                pf4 = pf[:].rearrange("p a (c f) -> p (a c) f", c=2)
                qkc = qkT_sb[tb]
                for qq in range(4):
                    nc.tensor.matmul(
                        pf4[:, qq, :], qkc[:, 128 * qq:128 * (qq + 1)],
                        projKZ_sb[:],
                        start=True, stop=True, skip_group_check=True)
                nc.scalar.activation(
                    ek[:, 4 * tb:4 * (tb + 1), :], pf4[:], AF.Exp, scale=DS)

            def emit_me_chunk(s, tb):
                ek = state[("ek", s)]
                me = state[("me", s)]
                nc.vector.tensor_reduce(
                    out=me[:, 4 * tb:4 * (tb + 1)],
                    in_=ek[:, 4 * tb:4 * (tb + 1), :],
                    axis=mybir.AxisListType.X, op=ALU.max)

            def emit_gq_chunk(s, c):
                # g = exp(-dn') / me, then vaug = [v * g | g]; 8-tile chunk c
                b, h = s
                knvn_sb = state[("knvn", b)]
                me = state[("me", s)]
                eg = state[("eg", s)]
                rme = state[("rme", s)]
                vaug = state[("vaug", s)]
                sl = slice(8 * c, 8 * (c + 1))
                nc.vector.reciprocal(rme[:, sl], me[:, sl])
                # write g straight into vaug's g-row (no separate copy)
                nc.vector.tensor_tensor(
                    out=vaug[:, sl, DH], in0=eg[:, sl], in1=rme[:, sl],
                    op=ALU.mult)
                nc.gpsimd.tensor_tensor(
                    out=vaug[:, sl, 0:DH],
                    in0=knvn_sb[c][:, :, 128 + DH * h:128 + DH * (h + 1)],
                    in1=vaug[:, sl, DH].rearrange("p (t one) -> p t one",
                                                  one=1)
                        .broadcast_to([128, 8, DH]),
                    op=ALU.mult)

            def emit_ctx_group(s, gi):
                # 4 ctx accumulation MMs (tiles 4*gi..4*gi+3) of slot s
                ek = state[("ek", s)]
                vaug = state[("vaug", s)]
                pctx = state[("pctx", s)]
                for t in range(4 * gi, 4 * gi + 4):
                    nc.tensor.matmul(
                        pctx[:], vaug[:, t, :], ek[:, t, :],
                        start=(t == 0), stop=(t == T - 1), skip_group_check=True)

            def emit_trans(s):
                pctx = state.pop(("pctx", s))
                ctxs = small_p.tile([65, 256], F32, tag="ctxs")
                nc.vector.tensor_copy(ctxs[:], pctx[:])
                # ctxT cols 0:64 = context dims, col 64 = kpsum, cols 65:128
                # = kpsum replicated -> the out MM emits the denominator on
                # partitions 64..127, so 1/den runs 64-lane-parallel with no
                # partition broadcast needed.
                ctxT = small_p.tile([128, 2, 128], BF16, tag="ctxT")
                pf_t = ps_mm.tile([128, 2, 512], F32, tag="mm", name="pf_t")
                for mi in range(2):
                    ptv = pf_t[:, mi, 0:65]
                    nc.tensor.transpose(ptv, ctxs[:, 128 * mi:128 * (mi + 1)],
                                        ident_sb[0:65, 0:65])
                    nc.scalar.activation(ctxT[:, mi, 0:65], ptv, AF.Copy)
                    nc.scalar.activation(
                        ctxT[:, mi, 65:128],
                        pf_t[:, mi, 64:65].broadcast_to([128, 63]), AF.Copy)
                state[("ctxT", s)] = ctxT

            def emit_qf(s, j):
                # q-feature MMs + exp for block j of slot s
                b, h = s
                qkT_sb = state[("qkT", b, h)]
                pq = ps_mm.tile([128, 2, 512], F32, tag="mm", name="pq")
                for mi in range(2):
                    nc.tensor.matmul(
                        pq[:, mi, :],
                        projQZ_sb[:, 128 * mi:128 * (mi + 1)],
                        qkT_sb[j][:],
                        start=True, stop=True, skip_group_check=True)
                qpt = qpt_p.tile([128, 2, 512], BF16, tag="qpt")
                nc.scalar.activation(qpt[:], pq[:], AF.Exp, scale=DS)
                state[("qpt", s, j)] = qpt

            def emit_out(s, j):
                b, h = s
                ctxT = state[("ctxT", s)]
                qpt = state.pop(("qpt", s, j))
                stg = state[("stg", s)]
                po = ps_po.tile([128, 512], F32, tag="po", name="po")
                for mi in range(2):
                    nc.tensor.matmul(
                        po[:], ctxT[:, mi, :], qpt[:, mi, :],
                        start=(mi == 0), stop=(mi == 1), skip_group_check=True)
                # normalize before shipping: rows 64.. of po hold the
                # (replicated) denominator
                den_sb = small_p.tile([64, 512], F32, tag="densb")
                nc.scalar.activation(den_sb[:], po[64:128, :], AF.Copy)
                rcp = small_p.tile([64, 512], F32, tag="rcp")
                nc.vector.reciprocal_approx_fast(rcp[:], den_sb[:])
                state[("ponrm", s, j)] = (po, rcp)

            def emit_out_mult(s, j):
                # deferred so the rcpB broadcast DMA latency stays off the
                # vector queue's critical path
                stg = state[("stg", s)]
                po, rcpB = state.pop(("ponrm", s, j))
                with nc.allow_low_precision("stg is shipped bf16 anyway"):
                    nc.vector.tensor_tensor(out=stg[:, j, :], in0=po[0:64, :],
                                            in1=rcpB[:], op=ALU.mult)

            def emit_slot_open(s):
                b, h = s
                ek = ek_p.tile([128, T, M], BF16, tag="ek", name="ek")
                state[("ek", s)] = ek
                state[("me", s)] = small_p.tile([128, T], F32, tag="me", name="me")
                state[("rme", s)] = small_p.tile([128, T], F32, tag="rme", name="rme")
                g_new = small_p.tile([128, T], BF16, tag="g", name="g")
                state[("g", s)] = g_new
                vaug = vaug_p.tile([128, T, 65], BF16, tag="vaug", name="vaug")
                state[("vaug", s)] = vaug
                knvn_sb = state[("knvn", b)]
                dn_h = small_p.tile([128, T], F32, tag="dn", name="dn_h")
                eg = small_p.tile([128, T], F32, tag="eg", name="eg")
                for c in range(4):
                    sl = slice(8 * c, 8 * (c + 1))
                    nc.vector.tensor_reduce(
                        out=dn_h[:, sl],
                        in_=knvn_sb[c][:, :, DH * h:DH * (h + 1)],
                        axis=mybir.AxisListType.X, op=ALU.add)
                    nc.scalar.activation(eg[:, sl], dn_h[:, sl], AF.Exp,
                                         scale=-0.5 * DS * DS)
                state[("eg", s)] = eg

            def emit_slot_close_prev(prev):
                # stg of prev is complete: ship it; after the second head
                # of a batch, trigger the batch's AllToAll (a half-size
                # per-head A2A runs at 1/4 the bus efficiency - not worth it)
                b, h = prev
                stg = state[("stg", prev)]
                nc.sync.dma_start(
                    out=h_in[b, :, h].rearrange("c p n -> p c n"), in_=stg[:])
                if h == 1:
                    nc.gpsimd.collective_compute(
                        "AllToAll", ALU.bypass, replica_groups=groups,
                        ins=[h_in[b]], outs=[h_out[b]])

            # ---- slot schedule ---------------------------------------
            # slot sigma = 2b + h.  In slot sigma we emit:
            #   kF(sigma) [scalar: ek exps], ctx(sigma-1), trans(sigma-1),
            #   qF(sigma-1) [scalar: qpt exps], out(sigma-1), lin hooks.
            slots = [(b, h) for b in range(B) for h in range(2)]

            def lin_hooks_for(sigma):
                # linear groups of batch bb: 4 at slot 2bb+4, 4 at 2bb+5
                for bb in range(B):
                    if sigma == 2 * bb + 4:
                        return [(bb, g) for g in range(4)]
                    if sigma == 2 * bb + 5:
                        return [(bb, g) for g in range(4, 8)]
                return []

            # -- head: first slot's inputs first, heavyweight consts later
            projKZ_sb = const_p.tile([128, M], BF16, tag="projKZ")
            nc.sync.dma_start(out=projKZ_sb[:], in_=projKZ[:])
            projQZ_sb = const_p.tile([128, M], BF16, tag="projQZ")
            nc.sync.dma_start(out=projQZ_sb[:], in_=projQZ[:])
            emit_qkT_load(0, 0, chunked=True)
            emit_knvn_load(0, chunked=True)
            ident_sb = const_p.tile([128, 128], F32, tag="ident")
            nc.sync.dma_start(out=ident_sb[:], in_=ident[:])
            WT_sb = const_p.tile([128, NCORES, D], BF16, tag="WT")
            nc.sync.dma_start(out=WT_sb[:], in_=WT[:])
            emit_ksq_dn(0)

            for sigma, s in enumerate(slots):
                b, h = s
                prev = slots[sigma - 1] if sigma > 0 else None
                if h == 0:
                    if b + 1 < B:
                        emit_knvn_load(b + 1)
                elif b + 1 < B:
                    emit_ksq_dn(b + 1)
                emit_slot_open(s)
                # prefetch next slot's qk tile (chunked for slot 1: its
                # load window is the congested startup slot 0)
                if sigma + 1 < len(slots):
                    emit_qkT_load(*slots[sigma + 1], chunked=(sigma == 0))
                if prev is not None:
                    state[("stg", prev)] = stg_p.tile(
                        [64, J, 512], BF16, tag="stg", name="stg")
                    state[("pctx", prev)] = ps_ctx.tile(
                        [65, 256], F32, tag="ctx", name="pctx")
                lins = lin_hooks_for(sigma)

                # interleave: kF steps + prev ctx groups + first prev qF
                for tb in range(8):
                    emit_kf_step(s, tb)
                    if prev is not None:
                        if tb >= 2:
                            emit_ctx_group(prev, tb - 2)
                        if tb == 5:
                            emit_qf(prev, 0)
                        if tb == 6:
                            emit_qf(prev, 1)
                    else:
                        # slot 0: vector is otherwise idle; me/gq inline
                        # so ctx(0) can start at slot 1's head
                        emit_me_chunk(s, tb)
                        if tb % 2 == 1:
                            emit_gq_chunk(s, tb // 2)
                if prev is not None:
                    emit_ctx_group(prev, 6)
                    emit_ctx_group(prev, 7)
                    emit_trans(prev)
                    for j in range(J):
                        if j + 2 < J:
                            emit_qf(prev, j + 2)
                        emit_out(prev, j)
                        if j > 0:
                            emit_out_mult(prev, j - 1)
                        if j < 4:
                            emit_me_chunk(s, 2 * j)
                            emit_me_chunk(s, 2 * j + 1)
                            emit_gq_chunk(s, j)
                        if j == 0 and b > 0 and h == 1:
                            # A2A(b-1) done by now -> build hgn(b-1)
                            emit_post_dma(b - 1)
                        if lins and j < len(lins):
                            emit_lin_group(*lins[j])
                    emit_out_mult(prev, J - 1)
                    emit_slot_close_prev(prev)


            # ---- tail: last slot's ctx/q/out + final linear ----------
            last = slots[-1]
            state[("stg", last)] = stg_p.tile([64, J, 512], BF16, tag="stg",
                                              name="stg")
            state[("pctx", last)] = ps_ctx.tile([65, 256], F32, tag="ctx",
                                                name="pctx")
            for gi in range(8):
                emit_ctx_group(last, gi)
            emit_trans(last)
            emit_qf(last, 0)
            emit_qf(last, 1)
            for j in range(J):
                if j + 2 < J:
                    emit_qf(last, j + 2)
                emit_out(last, j)
                if j > 0:
                    emit_out_mult(last, j - 1)
                # lin(2) groups 0-1 here; 2-7 stay behind the close to
                # cover the final AllToAll's latency
                if j < 2:
                    emit_lin_group(2, j)
            emit_out_mult(last, J - 1)
            emit_slot_close_prev(last)
            for g in range(2, 8):
                emit_lin_group(2, g)
            emit_post_dma(B - 1, dma_eng=nc.gpsimd)
            for g in range(8):
                emit_lin_group(B - 1, g)

    nc.compile()
    return nc


def _get_nc():
    if "nc" not in _CACHE:
        _CACHE["nc"] = _build()
    return _CACHE["nc"]


def _host_prep(q, k, v, W):
    qb = q.astype(NPBF16)
    kb = k.astype(NPBF16)
    vb = v.astype(NPBF16)
    # W.T rearranged: WT[p, cc, o] = W[o, cc*128 + p]
    WTh = np.ascontiguousarray(
        W.T.astype(NPBF16).reshape(NCORES, 128, D).transpose(1, 0, 2))
    identity = np.eye(128, dtype=np.float32)
    in_maps = []
    for c in range(NCORES):
        lo = c * 128
        qc = qb[:, :, lo:lo + 128]   # [B, N, 128]
        kc = kb[:, :, lo:lo + 128]
        vc = vb[:, :, lo:lo + 128]
        # [B, 2, 64, N] transposed per head-pair
        kT = kc.reshape(B, N, 2, DH).transpose(0, 2, 3, 1)
        qT = qc.reshape(B, N, 2, DH).transpose(0, 2, 3, 1)
        qkTh = np.ascontiguousarray(
            np.concatenate([kT, qT], axis=2))   # [B, 2, 128, N]
        kn = kc.reshape(B, T, 128, 128).transpose(0, 2, 1, 3)
        vn = vc.reshape(B, T, 128, 128).transpose(0, 2, 1, 3)
        knvnh = np.ascontiguousarray(
            np.concatenate([kn, vn], axis=3))   # [B, 128, T, 256]
        in_maps.append({
            "qkT": qkTh,
            "knvn": knvnh,
            "projKZ": None,   # filled below (shared)
            "projQZ": None,
            "WT": WTh,
            "ident": identity,
        })
    return in_maps


def kernel(q, k, v, W, b, proj, _profile=False):
    q = np.asarray(q, np.float32)
    k = np.asarray(k, np.float32)
    v = np.asarray(v, np.float32)
    W = np.asarray(W, np.float32)
    b = np.asarray(b, np.float32)
    proj = np.asarray(proj, np.float32)

    nc = _get_nc()
    in_maps = _host_prep(q, k, v, W)
    projT = np.ascontiguousarray(proj.T.astype(NPBF16))      # [64, M]
    zer = np.zeros_like(projT)
    projKZ = np.concatenate([projT, zer], axis=0)            # [128, M]
    projQZ = np.concatenate([zer, projT], axis=0)
    for m in in_maps:
        m["projKZ"] = projKZ
        m["projQZ"] = projQZ
    res = run_bass_kernel_spmd(nc, in_maps, list(range(NCORES)), trace=_profile)
    out = np.empty((B, N, D), dtype=np.float32)
    for c in range(NCORES):
        out[:, c * NS:(c + 1) * NS, :] = res.results[c]["out"]
    out += b
    if _profile:
        _CACHE["last_exec_time_ns"] = res.exec_time_ns
        _CACHE["last_profile_json"] = res.profile_json
    return out

